# revision 7
# baseline (speedup 1.0000x reference)
"""MultiHeadDiffAttn Trainium2 kernel (v3, row-tiled S + batched epilogue).

Sharding: 8 cores = 4-way data parallel over batch x 2-way tensor parallel
over heads (8 v-heads / 16 half-heads per core).  Each core computes its
batch's qkv projection restricted to its head group, differential attention
with per-half-head softmax, head RMS norm, and a partial output projection
(its 512 rows of w_proj).  Host sums the two partial projections per batch.

Key device-level choices:
  - all matmul operands are fp16 (fp32 streams at 1/4 rate); PSUM stays fp32.
  - S^T = k^T q per half-head contracts over only 32 dims, so it runs as
    K=32 row-tiled matmuls: the two halves of a v-head sit in different
    32-row groups of the PE array (tile_position) and stream concurrently,
    reading q/k 128-row chunks of the qkv output directly (no zero-padded
    scatter buffer).
  - S for both halves lives in one [128, 2, 1024] PSUM tile so exp is a
    single ACT op per (head, s-block); the causal mask is one affine_select
    over both halves on the idle GpSimd engine.
  - AV accumulates U[t-block, dv|den] in PSUM (4 t-strips per bank, ones
    column 64 = softmax denominator).  The s-loop is software-pipelined
    (AV groups one s-iteration late; head 0's first three s-blocks are
    prebaked into the qkv phase so ACT starts during the v projection).
  - the per-head epilogue is batched: one reciprocal over all 8 denominators
    per half, broadcast-AP multiplies over [128, 8, 64], and pool_avg for
    the RMS mean-square, instead of per-(head, t-tile) ops.
  - rstd = exp(-0.5 * ln(msq + eps)) keeps all ACT work in the single
    natural_log_exp table set (no mid-kernel table swaps).
  - outcat -> outcatT uses f16 DMA transposes alternating between the two
    HWDGE queues so the output projection streams without a serial tail.
"""

import math
from contextlib import ExitStack

import numpy as np

import concourse.bass as bass
import concourse.tile as tile
from concourse import masks, mybir
from concourse.bass_utils import run_bass_kernel_spmd

# The deployed walrus rejects instructions carrying more than one sync wait
# ("Too many sync wait commands" in setupSyncWait).  Legalize at the BIR-JSON
# level: for every instruction with >1 wait, hoist the extra waits onto NoOp
# instructions inserted just before it on the same engine (engine streams are
# in-order, so semantics are identical).
_MAX_WAITS = 1


def _legalize_sync_waits(d):
    for f in d.get("functions", []):
        for bb in f.get("blocks", []):
            out = []
            for inst in bb["instructions"]:
                si = inst.get("sync_info")
                waits = (si or {}).get("on_wait") or []
                if len(waits) > _MAX_WAITS:
                    extra = waits[: len(waits) - _MAX_WAITS]
                    keep = waits[len(waits) - _MAX_WAITS :]
                    for j in range(0, len(extra), _MAX_WAITS):
                        nop = {
                            "engine": inst["engine"],
                            "ins": [],
                            "outs": [],
                            "name": f"{inst['name']}-lw{j}",
                            "opcode": "NoOp",
                            "sync_info": {
                                "on_wait": extra[j : j + _MAX_WAITS],
                                "on_update": [],
                            },
                        }
                        if "debug" in inst:
                            nop["debug"] = inst["debug"]
                        out.append(nop)
                    si["on_wait"] = keep
                out.append(inst)
            bb["instructions"] = out
    return d


_orig_to_json_bytes = bass.Bass.to_json_bytes


def _patched_to_json_bytes(self, *a, **kw):
    import json as _json

    raw = _orig_to_json_bytes(self, *a, **kw)
    return _json.dumps(_legalize_sync_waits(_json.loads(raw))).encode()


bass.Bass.to_json_bytes = _patched_to_json_bytes

F32 = mybir.dt.float32
F16 = mybir.dt.float16

B, T, C = 4, 1024, 1024
H_TOT = 16  # total v-heads
HD = 32  # half-head dim
DV = 64  # v-head dim
G = 2  # head groups (tensor parallel)
HPG = H_TOT // G  # 8 v-heads per core
COLS = 1024  # q cols + k cols per group
LAMBDA_INIT = 0.8 - 0.6 * math.exp(-0.3 * (1 - 1))  # 0.2
EPS = 1e-5
N_CORES = 8

NT = T // 128  # 8 t-tiles
NKC = C // 128  # 8 contraction chunks
N_PREBAKE = 2  # head 0 s-blocks baked into the qkv phase


def _emit(ctx: ExitStack, tc: tile.TileContext, xT, w_qk, w_v, w_p, lam, y):
    nc = tc.nc
    AluOp = mybir.AluOpType
    Act = mybir.ActivationFunctionType

    const = ctx.enter_context(tc.tile_pool(name="const", bufs=1))
    ident = const.tile([128, 128], F16)
    masks.make_identity(nc, ident[:])
    lam_sb = const.tile([128, 1], F32)
    nc.sync.dma_start(out=lam_sb, in_=lam[:])
    eps_sb = const.tile([128, 1], F32)
    nc.vector.memset(eps_sb, EPS)

    big = ctx.enter_context(tc.tile_pool(name="big", bufs=1))
    qkT_sb = big.tile([128, 8, T], F16)  # row-chunks of [COLS, T]
    v_sb = big.tile([128, NT, HPG, DV + 1], F16)  # [s-chunk][head][dv | ones]
    outcat_sb = big.tile([128, NT, HPG * DV], F16)  # [t-chunk][512]
    outcatT_sb = big.tile([128, 4, T], F16)  # row-chunks of [512, T]
    wp_sb = big.tile([128, 4, C], F16)
    # combined per-head output (pre-RMS-scale), [t-part][tj][h*64+d]
    oh_sb = big.tile([128, NT, HPG * DV], F32)
    ssq_all = big.tile([128, HPG * NT], F32)  # mean(oh^2) per (h, tj)
    rstd_all = big.tile([128, HPG * NT], F32)

    es_pool = ctx.enter_context(tc.tile_pool(name="es", bufs=4))

    def emit_s_mms(h, s, ps_list, chunks):
        # S^T[s-block, t] per half: K=32 row-tiled matmuls, both halves
        # concurrent in different 32-row groups of the PE array.
        t0 = 128 * s
        c_ = h // 2
        for c0, c1 in chunks:
            for e in range(2):
                j = 2 * (h % 2) + e
                p0 = 32 * j
                nc.tensor.matmul(
                    ps_list[e][:, c0:c1],
                    qkT_sb[p0 : p0 + 32, 4 + c_, t0 : t0 + 128],
                    qkT_sb[p0 : p0 + 32, c_, c0:c1],
                    start=True,
                    stop=True,
                    tile_position=(p0, 0),
                )

    def emit_exp_mask(h, s, ps_ap_fn, es_t):
        # one exp over both halves' S rows, one causal mask for both halves
        t0 = 128 * s
        nc.scalar.activation(
            out=es_t[:, :, t0:T],
            in_=ps_ap_fn(t0),
            func=Act.Exp,
            scale=1.0 / 32.0,
        )
        nc.gpsimd.affine_select(
            out=es_t[:, :, t0 : t0 + 128],
            in_=es_t[:, :, t0 : t0 + 128],
            pattern=[[0, 2], [1, 128]],
            compare_op=AluOp.is_ge,
            fill=0.0,
            base=0,
            channel_multiplier=-1,
        )

    # ---------------- phase 1+2: qkv projections ----------------
    prebaked = []
    with (
        tc.tile_pool(name="xw", bufs=1) as xw,
        tc.tile_pool(name="mmps", bufs=4, space="PSUM") as mmps,
    ):
        xT_sb = xw.tile([128, NKC, T], F16)
        wqk_sb = xw.tile([128, NKC, COLS], F16)
        wv_sb = xw.tile([128, NKC, 512], F16)

        xT_r = xT[:].rearrange("(c p) t -> p c t", p=128)
        wqk_r = w_qk[:].rearrange("(c p) m -> p c m", p=128)

        CC_ORDER = [0, 4, 1, 2, 3, 5, 6, 7]

        def load_wqk(cc):
            nc.sync.dma_start(
                out=wqk_sb[:, :, cc * 128 : (cc + 1) * 128],
                in_=wqk_r[:, :, cc * 128 : (cc + 1) * 128],
            )

        load_wqk(0)
        load_wqk(4)
        for nh in range(2):  # t-halves so first matmuls start early
            nc.sync.dma_start(
                out=xT_sb[:, :, nh * 512 : (nh + 1) * 512],
                in_=xT_r[:, :, nh * 512 : (nh + 1) * 512],
            )
        for cc in CC_ORDER[2:]:
            load_wqk(cc)
        nc.sync.dma_start(out=wv_sb, in_=w_v[:].rearrange("(c p) m -> p c m", p=128))
        nc.sync.dma_start(
            out=wp_sb, in_=w_p[:].rearrange("(c p) m -> p c m", p=128)
        )

        # qkT[cc-block, :] = w_qk[:, cc-block].T @ x^T
        def emit_qk_chunk(cc):
            for nh in range(2):
                ps = mmps.tile([128, 1024], F32, tag="mmps", name=f"qk{cc}{nh}")[:, 0:512]
                for kc in range(NKC):
                    nc.tensor.matmul(
                        ps,
                        wqk_sb[:, kc, cc * 128 : (cc + 1) * 128],
                        xT_sb[:, kc, nh * 512 : (nh + 1) * 512],
                        start=(kc == 0),
                        stop=(kc == NKC - 1),
                    )
                nc.vector.tensor_copy(
                    out=qkT_sb[:, cc, nh * 512 : (nh + 1) * 512], in_=ps
                )

        emit_qk_chunk(0)
        emit_qk_chunk(4)

        # prebake head 0, s=0..N_PREBAKE-1: S/exp/mask run during the rest
        # of the qkv phase so ACT warms up early
        for s in range(N_PREBAKE):
            t0 = 128 * s
            chunks = [(t0, 512), (512, 1024)]
            ps_pair = [
                mmps.tile([128, 1024], F32, tag="mmps", name=f"sp{e}_{s}")
                for e in range(2)
            ]
            es_t = es_pool.tile([128, 2, T], F16, tag="es", name=f"esp_{s}")
            emit_s_mms(0, s, ps_pair, chunks)
            for e in range(2):
                nc.scalar.activation(
                    out=es_t[:, e, t0:T],
                    in_=ps_pair[e][:, t0:T],
                    func=Act.Exp,
                    scale=1.0 / 32.0,
                )
            nc.gpsimd.affine_select(
                out=es_t[:, :, t0 : t0 + 128],
                in_=es_t[:, :, t0 : t0 + 128],
                pattern=[[0, 2], [1, 128]],
                compare_op=AluOp.is_ge,
                fill=0.0,
                base=0,
                channel_multiplier=-1,
            )
            prebaked.append((s, es_t))

        for cc in CC_ORDER[2:]:
            emit_qk_chunk(cc)

        # v[t-block, :] = x @ w_v ; scatter heads into v_sb, slot 64 = ones
        for tt in range(NT):
            ps = mmps.tile([128, 1024], F32, tag="mmps", name=f"v{tt}")[:, 0:512]
            for kc in range(NKC):
                nc.tensor.matmul(
                    ps,
                    xT_sb[:, kc, tt * 128 : (tt + 1) * 128],
                    wv_sb[:, kc, :],
                    start=(kc == 0),
                    stop=(kc == NKC - 1),
                )
            nc.vector.tensor_copy(
                out=v_sb[:, tt, :, 0:DV],
                in_=ps[:].rearrange("p (h d) -> p h d", h=HPG),
            )
            nc.vector.memset(v_sb[:, tt, :, DV : DV + 1], 1.0)

    # ---------------- phase 3: differential attention ----------------
    with (
        tc.tile_pool(name="sps", bufs=1, space="PSUM") as s_pool,
        tc.tile_pool(name="ups", bufs=1, space="PSUM") as u_pool,
        tc.tile_pool(name="comb", bufs=2) as comb,
    ):
        def emit_av(h, s, es_t):
            # U[t-block, dv|den] += expS^T[s-block, t-block].T @ v_aug[s-block]
            for e in range(2):
                for tj in range(s, NT):
                    off = tj * 128
                    nc.tensor.matmul(
                        u_tiles[e][:, off : off + DV + 1],
                        es_t[:, e, off : off + 128],
                        v_sb[:, s, h, 0 : DV + 1],
                        start=(s == 0 and tj % 4 == 0),
                        stop=(s == tj and tj % 4 == 3),
                    )

        for h in range(HPG):
            s_tile = s_pool.tile([128, 2, 1024], F32, tag="s", name=f"s_{h}")
            u_tiles = [
                u_pool.tile([128, 1024], F32, tag=f"u{e}", name=f"u{e}_{h}")
                for e in range(2)
            ]
            if h == 0:
                for k in range(N_PREBAKE - 1):
                    emit_av(h, *prebaked[k])
                prev = prebaked[N_PREBAKE - 1]
                s_start = N_PREBAKE
            else:
                prev = None
                s_start = 0
            for s in range(s_start, NT):
                t0 = 128 * s
                chunks = [(t0, 512), (512, 1024)] if s < 4 else [(t0, 1024)]
                es_t = es_pool.tile([128, 2, T], F16, tag="es", name=f"es_{h}_{s}")
                emit_s_mms(h, s, [s_tile[:, 0], s_tile[:, 1]], chunks)
                if prev is not None:
                    emit_av(h, *prev)
                emit_exp_mask(h, s, lambda t0: s_tile[:, :, t0:T], es_t)
                prev = (s, es_t)
            emit_av(h, *prev)

            # ---- batched epilogue: normalize, lambda-combine, RMS stats ----
            u_r = [u_tiles[e][:].rearrange("p (j r) -> p j r", j=NT) for e in range(2)]
            rr_ = [
                comb.tile([128, NT], F32, tag=f"r{e}", name=f"r{e}_{h}")
                for e in range(2)
            ]
            for e in range(2):
                nc.vector.reciprocal(out=rr_[e], in_=u_r[e][:, :, DV : DV + 1])
            m0 = comb.tile([128, NT, DV], F32, tag="m0", name=f"m0_{h}")
            m1 = comb.tile([128, NT, DV], F32, tag="m1", name=f"m1_{h}")
            nc.vector.scalar_tensor_tensor(
                out=m1,
                in0=u_r[1][:, :, 0:DV],
                scalar=lam_sb[:],
                in1=rr_[1][:, :, None].broadcast_to([128, NT, DV]),
                op0=AluOp.mult,
                op1=AluOp.mult,
            )
            nc.vector.tensor_mul(
                m0, u_r[0][:, :, 0:DV], rr_[0][:, :, None].broadcast_to([128, NT, DV])
            )
            oh_ap = oh_sb[:, :, h * DV : (h + 1) * DV]
            nc.vector.tensor_sub(oh_ap, m0, m1)
            # pad the strip stride to DV+1 so the reduce input AP cannot be
            # collapsed to 2D (the X-axis reduction keys off the innermost dim)
            sq = comb.tile([128, NT, DV + 1], F32, tag="sq", name=f"sq_{h}")
            nc.vector.tensor_mul(sq[:, :, 0:DV], oh_ap, oh_ap)
            ssq_r = ssq_all[:].rearrange("p (h t) -> p h t", h=HPG)
            nc.vector.tensor_reduce(
                out=ssq_r[:, h, :],
                in_=sq[:, :, 0:DV],
                axis=mybir.AxisListType.X,
                op=AluOp.add,
            )

    # ---------------- phase 4: RMS scale + transpose ----------------
    # rstd = (msq + eps)^-0.5 via ln+exp: stays in the natural_log_exp
    # ACT table set, so the main exp stream never swaps tables.
    lnt = const.tile([128, HPG * NT], F32)
    nc.scalar.activation(
        out=lnt, in_=ssq_all[:], func=Act.Ln, bias=eps_sb[:], scale=1.0 / DV
    )
    nc.scalar.activation(out=rstd_all[:], in_=lnt, func=Act.Exp, scale=-0.5)
    rstd_r = rstd_all[:].rearrange("p (h t) -> p h t", h=HPG)
    for tj in range(NT):
        nc.vector.tensor_mul(
            outcat_sb[:, tj, :].rearrange("p (h d) -> p h d", h=HPG),
            oh_sb[:, tj, :].rearrange("p (h d) -> p h d", h=HPG),
            rstd_r[:, :, tj : tj + 1].broadcast_to([128, HPG, DV]),
        )
        eng = nc.sync
        eng.dma_start_transpose(
            out=outcatT_sb[:, :, tj * 128 : (tj + 1) * 128],
            in_=outcat_sb[:, tj, :],
        )

    # ---------------- phase 5: output projection ----------------
    with (
        tc.tile_pool(name="pps", bufs=4, space="PSUM") as pps,
        tc.tile_pool(name="yout", bufs=3) as yout,
    ):
        for tt in range(NT):
            yt = yout.tile([128, C], F32, tag="yt", name=f"y{tt}")
            for nh in range(2):
                ps = pps.tile([128, 512], F32, tag="pp", name=f"pp{tt}{nh}")
                for rr in range(4):
                    nc.tensor.matmul(
                        ps,
                        outcatT_sb[:, rr, tt * 128 : (tt + 1) * 128],
                        wp_sb[:, rr, nh * 512 : (nh + 1) * 512],
                        start=(rr == 0),
                        stop=(rr == 3),
                    )
                nc.vector.tensor_copy(out=yt[:, nh * 512 : (nh + 1) * 512], in_=ps)
            nc.sync.dma_start(out=y[tt * 128 : (tt + 1) * 128, :], in_=yt)


def build_nc():
    nc = bass.Bass()
    xT = nc.declare_dram_parameter("xT", [C, T], F16, isOutput=False)
    w_qk = nc.declare_dram_parameter("w_qk", [C, COLS], F16, isOutput=False)
    w_v = nc.declare_dram_parameter("w_v", [C, 512], F16, isOutput=False)
    w_p = nc.declare_dram_parameter("w_p", [512, C], F16, isOutput=False)
    lam = nc.declare_dram_parameter("lam", [128, 1], F32, isOutput=False)
    y = nc.declare_dram_parameter("y", [T, C], F32, isOutput=True)
    with tile.TileContext(nc) as tc:
        with ExitStack() as ctx:
            _emit(ctx, tc, xT, w_qk, w_v, w_p, lam, y)
    return nc


_NC = None


def _get_nc():
    global _NC
    if _NC is None:
        _NC = build_nc()
    return _NC


def make_in_maps(x, w_attn, w_proj, lambda_q1, lambda_q2, lambda_k1, lambda_k2, gamma):
    x = np.asarray(x, np.float32)
    w_attn = np.asarray(w_attn, np.float32)
    w_proj = np.asarray(w_proj, np.float32)
    lam1 = np.exp(np.sum(np.float32(lambda_q1) * np.float32(lambda_k1), dtype=np.float32))
    lam2 = np.exp(np.sum(np.float32(lambda_q2) * np.float32(lambda_k2), dtype=np.float32))
    lam_full = np.float32(lam1 - lam2 + LAMBDA_INIT)
    lam_tile = np.full((128, 1), lam_full, np.float32)
    # fold gamma * (1 - lambda_init) into w_proj rows
    scale = np.tile(np.asarray(gamma, np.float32), H_TOT) * np.float32(1.0 - LAMBDA_INIT)
    w_p_full = (w_proj * scale[:, None]).astype(np.float16)

    in_maps = []
    for core in range(N_CORES):
        b, g = core // G, core % G
        in_maps.append(
            {
                "xT": np.ascontiguousarray(x[b].T.astype(np.float16)),
                "w_qk": np.ascontiguousarray(
                    np.concatenate(
                        [
                            w_attn[:, g * 512 : (g + 1) * 512],
                            w_attn[:, C + g * 512 : C + (g + 1) * 512],
                        ],
                        axis=1,
                    ).astype(np.float16)
                ),
                "w_v": np.ascontiguousarray(
                    w_attn[:, 2 * C + g * 512 : 2 * C + (g + 1) * 512].astype(
                        np.float16
                    )
                ),
                "w_p": np.ascontiguousarray(w_p_full[g * 512 : (g + 1) * 512, :]),
                "lam": lam_tile,
            }
        )
    return in_maps


def assemble(results):
    y = np.empty((B, T, C), np.float32)
    for b in range(B):
        y[b] = results[b * G]["y"] + results[b * G + 1]["y"]
    return y


def kernel(**inputs) -> np.ndarray:
    nc = _get_nc()
    in_maps = make_in_maps(**inputs)
    res = run_bass_kernel_spmd(nc, in_maps, list(range(N_CORES)))
    return assemble(res.results)
